# revision 18
# baseline (speedup 1.0000x reference)
"""MoE FFN (8 experts, top-2) on 8 Trainium2 NeuronCores.

Strategy: expert parallelism with host-side token routing.
  - Host computes the (tiny) gate: logits = x @ gate_w.T, top-2, softmax.
  - Tokens are gathered per expert and padded to a common capacity C.
  - Core e runs a dense FFN (gelu(x@W1[e].T+b1[e])@W2[e].T+b2[e]) over the
    C tokens routed to expert e, all in one SPMD Bass program.
  - Host scatters y back with the combine weights and sums the two
    expert contributions per token.

Device kernel layout (per core):
  FFN1: psum[inter128, tok] += W1T[k*128:, m*128:].T @ xT[k*128:, tok]
        h = gelu(psum + b1)           (ACT, writes bf16)
  FFN2: psum[hid128, tok]  += W2T[k*128:, m*128:].T @ h[k*128:, tok]
        y = psum + b2                 (DVE, writes f32)
Weights held resident in SBUF as bf16; tokens stream in tiles of <=512.
"""

import sys
import types

import numpy as np
import ml_dtypes

import concourse.bass as bass
import concourse.tile as tile
from concourse import mybir
from concourse.bass_utils import run_bass_kernel_spmd
from bass_rust import ScopedClock, VectorClock


def _ensure_axon_hooks():
    """run_bass_kernel_spmd(trace=True) under axon imports antenv.axon_hooks,
    which this image's antenv lacks.  Register an equivalent module backed by
    trn_agent_boot's ctypes NTFF hook so tracing works (and trace=False paths
    are unaffected)."""
    try:
        import antenv.axon_hooks  # noqa: F401
        return
    except ImportError:
        pass
    hook = None
    try:
        from trn_agent_boot.trn_boot import _ntff_profile_via_ctypes
        hook = _ntff_profile_via_ctypes("/opt/axon/libaxon_pjrt.so")
    except Exception:
        hook = None
    mod = types.ModuleType("antenv.axon_hooks")
    _state = {"hook": hook}
    mod.get_axon_ntff_profile_hook = lambda: _state["hook"]
    mod.set_axon_ntff_profile_hook = lambda h: _state.__setitem__("hook", h)
    sys.modules["antenv.axon_hooks"] = mod
    try:
        import antenv
        antenv.axon_hooks = mod
    except ImportError:
        pass


_ensure_axon_hooks()

H = 1024          # hidden
I = 4096          # intermediate
E = 8             # experts
NCORES = 8
BF16 = mybir.dt.bfloat16
F32 = mybir.dt.float32


class _TC(tile.TileContext):
    """TileContext whose tail drain splits its sem waits across SP nops.

    The walrus pinned in this container rejects a Drain instruction carrying
    more than a couple of sync waits ("Too many sync wait commands",
    CoreV3GenImpl.cpp:104).  Emit one wait-carrier nop per logical processor
    instead, then a waitless drain.
    """

    def _drain_and_barrier(self, tick_clock, wait_clock):
        nc = self.nc
        gc = tick_clock.global_clock
        ticks = eval(repr(gc).replace("VectorClock(", "").rstrip(")"))
        for i, t in enumerate(ticks):
            if t > 0:
                partial = [0] * len(ticks)
                partial[i] = t
                carrier = nc.sync.nop(nofuse=True, hint=f"drain_wait_{i}")
                wait_clock.add_sem_waits(
                    carrier.ins, ScopedClock({None: VectorClock(partial)})
                )
        nc.sync.drain()
        nc.all_engine_barrier()
        assert self.sems is not None
        popped = nc._tile_sem_poison_stack.pop()
        assert popped is self._sem_poison
        nc.clear_and_free_semaphores(list(self.sems.allocated().values()))
        # No trailing all-engine barrier: every engine already passed the
        # barrier above, and NEFF completion waits for each engine's stream
        # (including the sem clears) anyway.  Saves ~2us of tail.


def _split_waits(nc, maxw=1):
    """The pinned walrus rejects instructions carrying more than one
    embedded sync wait ("Too many sync wait commands").  Hoist excess waits
    onto freshly inserted same-engine nops placed directly before the
    instruction — the engine sequencer executes them in order, so the
    semantics are identical."""
    for fn in nc.m.functions:
        for bb in fn.blocks:
            new = []
            changed = False
            for inst in bb.instructions:
                si = inst.sync_info
                waits = list(si.on_wait) if si is not None else []
                if len(waits) > maxw:
                    changed = True
                    n_extra = len(waits) - maxw
                    for i in range(0, n_extra, maxw):
                        nop = mybir.InstNoOp(
                            name=nc.get_next_instruction_name(),
                            engine=inst.engine,
                            sync_info=mybir.SyncInfo(
                                on_wait=waits[i:i + maxw], on_update=[]
                            ),
                            bass_nofuse=True,
                        )
                        nc.register_instruction(nop, overwrite=True)
                        new.append(nop)
                    si.on_wait = waits[n_extra:]
                new.append(inst)
            if changed:
                bb.instructions = new


def _token_tiles(C):
    # Remainder tile last: the first (full) tile's FFN1 masks the W2 load.
    tiles = [512] * (C // 512)
    if C % 512:
        tiles.append(C % 512)
    return tiles


def _build(C):
    """Dense per-expert FFN over C tokens; one SPMD program for all cores."""
    KH = H // 128   # 8  k-tiles over hidden
    KI = I // 128   # 32 k-tiles over inter
    nc = bass.Bass()
    xt = nc.declare_dram_parameter("xt", [H, C], BF16, isOutput=False)
    w1t = nc.declare_dram_parameter("w1t", [H, I], BF16, isOutput=False)
    w2t = nc.declare_dram_parameter("w2t", [I, H], BF16, isOutput=False)
    b1 = nc.declare_dram_parameter("b1", [128, KI], F32, isOutput=False)
    b2 = nc.declare_dram_parameter("b2", [128, KH], F32, isOutput=False)
    yt = nc.declare_dram_parameter("yt", [H, C], F32, isOutput=True)

    with _TC(nc) as tc:
        with (
            tc.tile_pool(name="weights", bufs=1) as wpool,
            tc.tile_pool(name="bias", bufs=1) as bpool,
            tc.tile_pool(name="x", bufs=3) as xpool,
            tc.tile_pool(name="h", bufs=1) as hpool,
            tc.tile_pool(name="o", bufs=4) as opool,
            tc.tile_pool(name="ps1", bufs=4, space="PSUM") as ps1pool,
            tc.tile_pool(name="ps2", bufs=4, space="PSUM") as ps2pool,
        ):
            # Latency-critical small loads on GpSimd SWDGE queues so they
            # don't queue behind the 16 MB of weight traffic on the sync
            # HWDGE queues.
            b1s = bpool.tile([128, KI], F32, tag="b1")
            nc.gpsimd.dma_start(b1s[:], b1[:])
            b2s = bpool.tile([128, KH], F32, tag="b2")
            nc.gpsimd.dma_start(b2s[:], b2[:])
            # W1 in column phases: phase 0 covers the first m-blocks of all
            # k-tiles, so FFN1 can start after ~2 MB instead of ~8 MB.
            w1s = [
                wpool.tile([128, I], BF16, tag=f"w1_{k}", name=f"w1_{k}")
                for k in range(KH)
            ]
            # Fine 128-col phases for the first 1024 cols: the PE consumes
            # (m,k) weight blocks m-minor, so each m-block needs that
            # column range of ALL k-tiles — small early phases keep the
            # delivery ahead of the ~1.7us/m-block consumption rate.
            bounds = [128 * i for i in range(9)] + [1024 * i for i in range(2, 5)]
            for lo, hi in zip(bounds[:-1], bounds[1:]):
                for k in range(KH):
                    nc.sync.dma_start(
                        w1s[k][:, lo:hi], w1t[k * 128:(k + 1) * 128, lo:hi]
                    )
            # W2 afterwards, in FFN2 consumption order (k ascending).
            w2s = []
            for k in range(KI):
                w = wpool.tile([128, H], BF16, tag=f"w2_{k}")
                nc.sync.dma_start(w[:], w2t[k * 128:(k + 1) * 128, :])
                w2s.append(w)

            off = 0
            for ti, tw in enumerate(_token_tiles(C)):
                xs = xpool.tile([128, KH * tw], BF16, tag="xt")
                # First tile: halve each chunk so the 8 SWDGE queues turn
                # around faster and the first psum-group unblocks sooner.
                nsplit = 2 if ti == 0 else 1
                for k in range(KH):
                    step = tw // nsplit
                    for s in range(nsplit):
                        nc.gpsimd.dma_start(
                            xs[:, k * tw + s * step:k * tw + (s + 1) * step],
                            xt[k * 128:(k + 1) * 128,
                               off + s * step:off + (s + 1) * step],
                        )
                ht = hpool.tile([128, KI * tw], BF16, tag="h")
                for m in range(KI):
                    ps = ps1pool.tile([128, tw], F32, tag="ps1")
                    for k in range(KH):
                        nc.tensor.matmul(
                            ps[:],
                            w1s[k][:, m * 128:(m + 1) * 128],
                            xs[:, k * tw:(k + 1) * tw],
                            start=(k == 0),
                            stop=(k == KH - 1),
                        )
                    nc.scalar.activation(
                        ht[:, m * tw:(m + 1) * tw],
                        ps[:],
                        mybir.ActivationFunctionType.Gelu,
                        bias=b1s[:, m:m + 1],
                    )
                for m in range(KH):
                    ps = ps2pool.tile([128, tw], F32, tag="ps2")
                    for k in range(KI):
                        nc.tensor.matmul(
                            ps[:],
                            w2s[k][:, m * 128:(m + 1) * 128],
                            ht[:, k * tw:(k + 1) * tw],
                            start=(k == 0),
                            stop=(k == KI - 1),
                        )
                    ot = opool.tile([128, tw], F32, tag="o")
                    nc.vector.tensor_scalar_add(ot[:], ps[:], b2s[:, m:m + 1])
                    nc.scalar.dma_start(
                        yt[m * 128:(m + 1) * 128, off:off + tw], ot[:]
                    )
                off += tw
    _split_waits(nc)
    return nc


def _route(x, gate_w):
    """Host gate: top-2 of 8 logits + softmax over the selected pair."""
    logits = x @ gate_w.T                         # [T, E] f32
    T = logits.shape[0]
    rows = np.arange(T)
    i1 = np.argmax(logits, axis=1)
    v1 = logits[rows, i1]
    masked = logits.copy()
    masked[rows, i1] = -np.inf
    i2 = np.argmax(masked, axis=1)
    v2 = masked[rows, i2]
    # softmax over (v1, v2) with v1 >= v2
    e2 = np.exp(v2 - v1)
    w1 = 1.0 / (1.0 + e2)
    w2 = 1.0 - w1
    return i1, i2, w1.astype(np.float32), w2.astype(np.float32)


def _run(inputs, trace=False):
    hidden_states = np.asarray(inputs["hidden_states"], dtype=np.float32)
    gate_w = np.asarray(inputs["gate_w"], dtype=np.float32)
    W1 = np.asarray(inputs["W1"], dtype=np.float32)
    b1 = np.asarray(inputs["b1"], dtype=np.float32)
    W2 = np.asarray(inputs["W2"], dtype=np.float32)
    b2 = np.asarray(inputs["b2"], dtype=np.float32)

    B, S, _ = hidden_states.shape
    T = B * S
    x = np.ascontiguousarray(hidden_states.reshape(T, H))

    i1, i2, w1, w2 = _route(x, gate_w)
    toks = [np.flatnonzero((i1 == e) | (i2 == e)) for e in range(E)]
    cnts = [len(t) for t in toks]
    C = max(128, -(-max(cnts) // 128) * 128)

    nc = _build(C)

    in_maps = []
    for e in range(E):
        xe = np.zeros((C, H), dtype=ml_dtypes.bfloat16)
        xe[: cnts[e]] = x[toks[e]].astype(ml_dtypes.bfloat16)
        in_maps.append(
            {
                "xt": np.ascontiguousarray(xe.T),
                "w1t": np.ascontiguousarray(W1[e].astype(ml_dtypes.bfloat16).T),
                "w2t": np.ascontiguousarray(W2[e].astype(ml_dtypes.bfloat16).T),
                "b1": np.ascontiguousarray(b1[e].reshape(I // 128, 128).T),
                "b2": np.ascontiguousarray(b2[e].reshape(H // 128, 128).T),
            }
        )

    res = run_bass_kernel_spmd(
        nc, in_maps, core_ids=list(range(NCORES)), trace=trace
    )

    out = np.zeros((T, H), dtype=np.float32)
    for e in range(E):
        te = toks[e]
        ye = res.results[e]["yt"][:, : cnts[e]].T          # [cnt, H]
        we = np.where(i1[te] == e, w1[te], w2[te])
        out[te] += we[:, None] * ye
    return out.reshape(B, S, H), res


def kernel(**inputs):
    out, _ = _run(inputs, trace=False)
    return out


# revision 20
# speedup vs baseline: 1.0346x; 1.0346x over previous
"""MoE FFN (8 experts, top-2) on 8 Trainium2 NeuronCores.

Strategy: expert parallelism with host-side token routing.
  - Host computes the (tiny) gate: logits = x @ gate_w.T, top-2, softmax.
  - Tokens are gathered per expert and padded to a common capacity C.
  - Core e runs a dense FFN (gelu(x@W1[e].T+b1[e])@W2[e].T+b2[e]) over the
    C tokens routed to expert e, all in one SPMD Bass program.
  - Host scatters y back with the combine weights and sums the two
    expert contributions per token.

Device kernel layout (per core):
  FFN1: psum[inter128, tok] += W1T[k*128:, m*128:].T @ xT[k*128:, tok]
        h = gelu(psum + b1)           (ACT, writes bf16)
  FFN2: psum[hid128, tok]  += W2T[k*128:, m*128:].T @ h[k*128:, tok]
        y = psum + b2                 (DVE, writes f32)
Weights held resident in SBUF as bf16; tokens stream in tiles of <=512.
"""

import sys
import types

import numpy as np
import ml_dtypes

import concourse.bass as bass
import concourse.tile as tile
from concourse import mybir
from concourse.bass_utils import run_bass_kernel_spmd
from bass_rust import ScopedClock, VectorClock


def _ensure_axon_hooks():
    """run_bass_kernel_spmd(trace=True) under axon imports antenv.axon_hooks,
    which this image's antenv lacks.  Register an equivalent module backed by
    trn_agent_boot's ctypes NTFF hook so tracing works (and trace=False paths
    are unaffected)."""
    try:
        import antenv.axon_hooks  # noqa: F401
        return
    except ImportError:
        pass
    hook = None
    try:
        from trn_agent_boot.trn_boot import _ntff_profile_via_ctypes
        hook = _ntff_profile_via_ctypes("/opt/axon/libaxon_pjrt.so")
    except Exception:
        hook = None
    mod = types.ModuleType("antenv.axon_hooks")
    _state = {"hook": hook}
    mod.get_axon_ntff_profile_hook = lambda: _state["hook"]
    mod.set_axon_ntff_profile_hook = lambda h: _state.__setitem__("hook", h)
    sys.modules["antenv.axon_hooks"] = mod
    try:
        import antenv
        antenv.axon_hooks = mod
    except ImportError:
        pass


_ensure_axon_hooks()

H = 1024          # hidden
I = 4096          # intermediate
E = 8             # experts
NCORES = 8
BF16 = mybir.dt.bfloat16
F32 = mybir.dt.float32


class _TC(tile.TileContext):
    """TileContext whose tail drain splits its sem waits across SP nops.

    The walrus pinned in this container rejects a Drain instruction carrying
    more than a couple of sync waits ("Too many sync wait commands",
    CoreV3GenImpl.cpp:104).  Emit one wait-carrier nop per logical processor
    instead, then a waitless drain.
    """

    def _drain_and_barrier(self, tick_clock, wait_clock):
        nc = self.nc
        gc = tick_clock.global_clock
        ticks = eval(repr(gc).replace("VectorClock(", "").rstrip(")"))
        for i, t in enumerate(ticks):
            if t > 0:
                partial = [0] * len(ticks)
                partial[i] = t
                carrier = nc.sync.nop(nofuse=True, hint=f"drain_wait_{i}")
                wait_clock.add_sem_waits(
                    carrier.ins, ScopedClock({None: VectorClock(partial)})
                )
        nc.sync.drain()
        nc.all_engine_barrier()
        assert self.sems is not None
        popped = nc._tile_sem_poison_stack.pop()
        assert popped is self._sem_poison
        nc.clear_and_free_semaphores(list(self.sems.allocated().values()))
        nc.all_engine_barrier()


def _split_waits(nc, maxw=1):
    """The pinned walrus rejects instructions carrying more than one
    embedded sync wait ("Too many sync wait commands").  Hoist excess waits
    onto freshly inserted same-engine nops placed directly before the
    instruction — the engine sequencer executes them in order, so the
    semantics are identical."""
    for fn in nc.m.functions:
        for bb in fn.blocks:
            new = []
            changed = False
            for inst in bb.instructions:
                si = inst.sync_info
                waits = list(si.on_wait) if si is not None else []
                if len(waits) > maxw:
                    changed = True
                    n_extra = len(waits) - maxw
                    for i in range(0, n_extra, maxw):
                        nop = mybir.InstNoOp(
                            name=nc.get_next_instruction_name(),
                            engine=inst.engine,
                            sync_info=mybir.SyncInfo(
                                on_wait=waits[i:i + maxw], on_update=[]
                            ),
                            bass_nofuse=True,
                        )
                        nc.register_instruction(nop, overwrite=True)
                        new.append(nop)
                    si.on_wait = waits[n_extra:]
                new.append(inst)
            if changed:
                bb.instructions = new


def _token_tiles(C):
    # Remainder tile last: the first (full) tile's FFN1 masks the W2 load.
    tiles = [512] * (C // 512)
    if C % 512:
        tiles.append(C % 512)
    return tiles


def _build(C):
    """Dense per-expert FFN over C tokens; one SPMD program for all cores."""
    KH = H // 128   # 8  k-tiles over hidden
    KI = I // 128   # 32 k-tiles over inter
    nc = bass.Bass()
    xt = nc.declare_dram_parameter("xt", [H, C], BF16, isOutput=False)
    w1t = nc.declare_dram_parameter("w1t", [H, I], BF16, isOutput=False)
    w2t = nc.declare_dram_parameter("w2t", [I, H], BF16, isOutput=False)
    b1 = nc.declare_dram_parameter("b1", [128, KI], F32, isOutput=False)
    b2 = nc.declare_dram_parameter("b2", [128, KH], F32, isOutput=False)
    yt = nc.declare_dram_parameter("yt", [H, C], F32, isOutput=True)

    with _TC(nc) as tc:
        with (
            tc.tile_pool(name="weights", bufs=1) as wpool,
            tc.tile_pool(name="bias", bufs=1) as bpool,
            tc.tile_pool(name="x", bufs=3) as xpool,
            tc.tile_pool(name="h", bufs=1) as hpool,
            tc.tile_pool(name="o", bufs=4) as opool,
            tc.tile_pool(name="ps1", bufs=4, space="PSUM") as ps1pool,
            tc.tile_pool(name="ps2", bufs=4, space="PSUM") as ps2pool,
        ):
            # Latency-critical small loads on GpSimd SWDGE queues so they
            # don't queue behind the 16 MB of weight traffic on the sync
            # HWDGE queues.
            b1s = bpool.tile([128, KI], F32, tag="b1")
            nc.gpsimd.dma_start(b1s[:], b1[:])
            b2s = bpool.tile([128, KH], F32, tag="b2")
            nc.gpsimd.dma_start(b2s[:], b2[:])
            # W1 in column phases: phase 0 covers the first m-blocks of all
            # k-tiles, so FFN1 can start after ~2 MB instead of ~8 MB.
            w1s = [
                wpool.tile([128, I], BF16, tag=f"w1_{k}", name=f"w1_{k}")
                for k in range(KH)
            ]
            # Small first phase (256 cols = 512 KB) so the first FFN1
            # psum-groups unblock early, then coarse 960-col phases.
            # (Finer 128-col phases measured WORSE: 64 small descriptors
            # slow the aggregate delivery and triple the startup stalls.)
            bounds = [0, 256] + [256 + 960 * i for i in range(1, 5)]
            for lo, hi in zip(bounds[:-1], bounds[1:]):
                for k in range(KH):
                    nc.sync.dma_start(
                        w1s[k][:, lo:hi], w1t[k * 128:(k + 1) * 128, lo:hi]
                    )
            # W2 afterwards, in FFN2 consumption order (k ascending).
            w2s = []
            for k in range(KI):
                w = wpool.tile([128, H], BF16, tag=f"w2_{k}")
                nc.sync.dma_start(w[:], w2t[k * 128:(k + 1) * 128, :])
                w2s.append(w)

            off = 0
            for ti, tw in enumerate(_token_tiles(C)):
                xs = xpool.tile([128, KH * tw], BF16, tag="xt")
                # First tile: halve each chunk so the 8 SWDGE queues turn
                # around faster and the first psum-group unblocks sooner.
                nsplit = 2 if ti == 0 else 1
                for k in range(KH):
                    step = tw // nsplit
                    for s in range(nsplit):
                        nc.gpsimd.dma_start(
                            xs[:, k * tw + s * step:k * tw + (s + 1) * step],
                            xt[k * 128:(k + 1) * 128,
                               off + s * step:off + (s + 1) * step],
                        )
                ht = hpool.tile([128, KI * tw], BF16, tag="h")
                for m in range(KI):
                    ps = ps1pool.tile([128, tw], F32, tag="ps1")
                    for k in range(KH):
                        nc.tensor.matmul(
                            ps[:],
                            w1s[k][:, m * 128:(m + 1) * 128],
                            xs[:, k * tw:(k + 1) * tw],
                            start=(k == 0),
                            stop=(k == KH - 1),
                        )
                    nc.scalar.activation(
                        ht[:, m * tw:(m + 1) * tw],
                        ps[:],
                        mybir.ActivationFunctionType.Gelu,
                        bias=b1s[:, m:m + 1],
                    )
                for m in range(KH):
                    ps = ps2pool.tile([128, tw], F32, tag="ps2")
                    for k in range(KI):
                        nc.tensor.matmul(
                            ps[:],
                            w2s[k][:, m * 128:(m + 1) * 128],
                            ht[:, k * tw:(k + 1) * tw],
                            start=(k == 0),
                            stop=(k == KI - 1),
                        )
                    ot = opool.tile([128, tw], F32, tag="o")
                    nc.vector.tensor_scalar_add(ot[:], ps[:], b2s[:, m:m + 1])
                    nc.scalar.dma_start(
                        yt[m * 128:(m + 1) * 128, off:off + tw], ot[:]
                    )
                off += tw
    _split_waits(nc)
    return nc


def _route(x, gate_w):
    """Host gate: top-2 of 8 logits + softmax over the selected pair."""
    logits = x @ gate_w.T                         # [T, E] f32
    T = logits.shape[0]
    rows = np.arange(T)
    i1 = np.argmax(logits, axis=1)
    v1 = logits[rows, i1]
    masked = logits.copy()
    masked[rows, i1] = -np.inf
    i2 = np.argmax(masked, axis=1)
    v2 = masked[rows, i2]
    # softmax over (v1, v2) with v1 >= v2
    e2 = np.exp(v2 - v1)
    w1 = 1.0 / (1.0 + e2)
    w2 = 1.0 - w1
    return i1, i2, w1.astype(np.float32), w2.astype(np.float32)


def _run(inputs, trace=False):
    hidden_states = np.asarray(inputs["hidden_states"], dtype=np.float32)
    gate_w = np.asarray(inputs["gate_w"], dtype=np.float32)
    W1 = np.asarray(inputs["W1"], dtype=np.float32)
    b1 = np.asarray(inputs["b1"], dtype=np.float32)
    W2 = np.asarray(inputs["W2"], dtype=np.float32)
    b2 = np.asarray(inputs["b2"], dtype=np.float32)

    B, S, _ = hidden_states.shape
    T = B * S
    x = np.ascontiguousarray(hidden_states.reshape(T, H))

    i1, i2, w1, w2 = _route(x, gate_w)
    toks = [np.flatnonzero((i1 == e) | (i2 == e)) for e in range(E)]
    cnts = [len(t) for t in toks]
    C = max(128, -(-max(cnts) // 128) * 128)

    nc = _build(C)

    in_maps = []
    for e in range(E):
        xe = np.zeros((C, H), dtype=ml_dtypes.bfloat16)
        xe[: cnts[e]] = x[toks[e]].astype(ml_dtypes.bfloat16)
        in_maps.append(
            {
                "xt": np.ascontiguousarray(xe.T),
                "w1t": np.ascontiguousarray(W1[e].astype(ml_dtypes.bfloat16).T),
                "w2t": np.ascontiguousarray(W2[e].astype(ml_dtypes.bfloat16).T),
                "b1": np.ascontiguousarray(b1[e].reshape(I // 128, 128).T),
                "b2": np.ascontiguousarray(b2[e].reshape(H // 128, 128).T),
            }
        )

    res = run_bass_kernel_spmd(
        nc, in_maps, core_ids=list(range(NCORES)), trace=trace
    )

    out = np.zeros((T, H), dtype=np.float32)
    for e in range(E):
        te = toks[e]
        ye = res.results[e]["yt"][:, : cnts[e]].T          # [cnt, H]
        we = np.where(i1[te] == e, w1[te], w2[te])
        out[te] += we[:, None] * ye
    return out.reshape(B, S, H), res


def kernel(**inputs):
    out, _ = _run(inputs, trace=False)
    return out
